# revision 1
# baseline (speedup 1.0000x reference)
"""DETR3D decoder layer on 8 Trainium2 NeuronCores (Bass/Tile).

Sharding: 2 batches x 4 cores.  Within each batch-group of 4 cores:
  - queries padded 900->1024 and q-sharded 256/core (self-attn, FFN, pos-enc;
    K/V projections replicated within the group),
  - cross-attn feature maps sharded by (camera, 128-channel-chunk) "units",
    3 units per core (6 cams x 2 chunks = 12 units / 4 cores),
  - collectives: AllGather of the attention-weight rows, ReduceScatter of the
    camera-fused feature sum.  Groups of 4 (one per batch).
Per-core pipeline: feature sweep (HWDGE load -> PE transpose -> bf16
channel-last pyramid resident in SBUF) overlapped with self-attention;
dma_gather (SBUF source) of the 4 bilinear corners; coefficient-weighted
combine; ReduceScatter; out-proj + position-encoder + FFN tail.
"""

import math
import numpy as np

B, Q, C, NH, DH, NCAM, NL, FFD = 2, 900, 256, 8, 32, 6, 3, 1024
QP = 1024          # padded queries per batch
QL = 256           # queries per core
LVLS = [(116, 200), (58, 100), (29, 50)]
HWL = [h * w for h, w in LVLS]             # 23200, 5800, 1450
BLKS = [(hw + 127) // 128 for hw in HWL]   # 182, 46, 12
IMG_H, IMG_W = 928.0, 1600.0
PCS = [102.4, 102.4, 8.0]
PCO = [-51.2, -51.2, -5.0]
EPS = 1e-5
NU = 3             # units per core
NJ = NU * NL * 4   # index groups: ((u*NL+l)*4 + corner)
NCORES = 8
GROUPS = [[0, 1, 2, 3], [4, 5, 6, 7]]

_BUILT = {}
FLAGS = {"gather": True, "coll": True, "sweep": True, "dbg": False, "comb": True}


def _build():
    import concourse.bass as bass
    import concourse.bacc as bacc
    import concourse.tile as tile
    from concourse import mybir

    nc = bacc.Bacc("TRN2", target_bir_lowering=False, debug=False,
                   num_devices=NCORES)
    with tile.TileContext(nc) as tc:
        _emit(tc, nc, bass, mybir)
    nc.compile()
    return nc


def _emit(tc, nc, bass, mybir):
    import contextlib
    F32 = mybir.dt.float32
    BF = mybir.dt.bfloat16
    I16 = mybir.dt.int16
    AF = mybir.ActivationFunctionType
    AL = mybir.AluOpType
    AX = mybir.AxisListType
    ts_ = bass.ts
    AWC = NCAM * NL   # 18

    def din(name, shape, dt=F32):
        return nc.dram_tensor(name, list(shape), dt, kind="ExternalInput").ap()

    # ------------- external inputs -------------
    xposbf_d = din("xposbf", (128, 2, QP), BF)
    xvbf_d = din("xvbf", (128, 2, QP), BF)
    xlocbf_d = din("xlocbf", (128, 2, QL), BF)
    qrows_d = din("qrows_loc", (QL, C))
    refs_d = din("refs", (3, QP))
    refrows_d = din("refrows_loc", (QL, 3))
    lidarT_d = din("lidarT", (4, 4 * NCAM))
    camsel_d = din("camsel", (128, NU, NCAM))
    chsel_d = din("chsel", (128, NU, 2))
    feat_d = [din(f"feat{l}", (NU, 128, LVLS[l][0], LVLS[l][1]))
              for l in range(NL)]
    wqkvT_d = din("wqkvT", (C, 3 * C), BF)
    woT_d = din("woT", (C, C), BF)
    attWT_d = din("attWT", (C, AWC), BF)
    outWT_d = din("outWT", (C, C), BF)
    peW1T_d = din("peW1T", (3, C), BF)
    peW2T_d = din("peW2T", (C, C), BF)
    ffW1T_d = din("ffW1T", (C, FFD), BF)
    ffW2T_d = din("ffW2T", (FFD, C), BF)
    bq_d = din("bq", (128, 2))
    bk_d = din("bk", (128, 2))
    bv_d = din("bv_b", (128, C))
    bo_d = din("bo_b", (128, C))
    attb_d = din("attb_b", (128, AWC))
    outb_d = din("outb_b", (128, C))
    peb1_d = din("peb1_b", (128, C))
    peb2_d = din("peb2_b", (128, C))
    ffb1_d = din("ffb1", (128, 8))
    ffb2_d = din("ffb2_b", (128, C))
    nrm_d = {k: din(k + "_b", (128, C)) for k in
             ("n1g", "n1b", "n2g", "n2b", "n3g", "n3b",
              "peg1", "pebt1", "peg2", "pebt2")}
    identf_d = din("identf", (128, 128))
    ones1_d = din("ones1", (1, 128), BF)
    onesf_d = din("onesf", (1, 128))

    out_rows = nc.dram_tensor("out_rows", [QL, C], F32,
                              kind="ExternalOutput").ap()
    if FLAGS["dbg"]:
        dbg_idx = nc.dram_tensor("dbg_idx", [128, NJ * 64], I16,
                                 kind="ExternalOutput").ap()
        dbg_cof = nc.dram_tensor("dbg_cof", [NJ // 4, 4096], BF,
                                 kind="ExternalOutput").ap()
    aw_in = nc.dram_tensor("aw_in", [QL, AWC], F32, kind="Internal").ap()
    aw_all = nc.dram_tensor("aw_all", [QP, AWC], F32,
                            kind="Internal").ap()
    fs_in = nc.dram_tensor("fs_in", [QP, C], F32, kind="Internal").ap()
    fs_out = nc.dram_tensor("fs_out", [QL, C], F32,
                            kind="Internal").ap()
    idx_d = nc.dram_tensor("idx_d", [16, NJ * 64], I16, kind="Internal").ap()
    cof_d = nc.dram_tensor("cof_d", [NJ // 4, 4096], BF, kind="Internal").ap()

    ctx = contextlib.ExitStack()
    with ctx:
        # ------------- pools -------------
        cons = ctx.enter_context(tc.tile_pool(name="cons", bufs=1))
        work = ctx.enter_context(tc.tile_pool(name="work", bufs=3))
        stage_p = ctx.enter_context(tc.tile_pool(name="stage", bufs=2))
        pyr_p = [ctx.enter_context(tc.tile_pool(name=f"pyr{l}", bufs=1))
                 for l in range(NL)]
        g_p = ctx.enter_context(tc.tile_pool(name="gat", bufs=2))
        crow_p = ctx.enter_context(tc.tile_pool(name="crow", bufs=2))
        esc_p = ctx.enter_context(tc.tile_pool(name="esc", bufs=2))
        ht_p = ctx.enter_context(tc.tile_pool(name="ht", bufs=3))
        tmp_p = ctx.enter_context(tc.tile_pool(name="tmpc", bufs=2))
        ps_tp = ctx.enter_context(tc.tile_pool(name="ps_tp", bufs=3,
                                               space="PSUM"))
        ps_mm = ctx.enter_context(tc.tile_pool(name="ps_mm", bufs=3,
                                               space="PSUM"))
        ps_bc = ctx.enter_context(tc.tile_pool(name="ps_bc", bufs=2,
                                               space="PSUM"))

        MM = nc.tensor.matmul
        bias_p = ctx.enter_context(tc.tile_pool(name="biasp", bufs=4))
        wz_p = ctx.enter_context(tc.tile_pool(name="wzp", bufs=2))

        def lb(ap):
            t = bias_p.tile(list(ap.shape), ap.dtype, name="lb", tag="lb")
            nc.sync.dma_start(out=t[:], in_=ap[:])
            return t

        def lw(ap):
            rows, ncols = ap.shape
            k = rows // 128
            t = wz_p.tile([128, k, ncols], ap.dtype, name="lw", tag="lw")
            srcap = bass.AP(tensor=ap.tensor, offset=0,
                            ap=[[ncols, 128], [128 * ncols, k], [1, ncols]])
            nc.sync.dma_start(out=t[:], in_=srcap)
            return t

        def csb(shape, dt, tag):       # persistent tile (unique tag!)
            return cons.tile(list(shape), dt, name=tag, tag=tag)

        def load(ap, tag):
            t = csb(ap.shape, ap.dtype, tag)
            nc.sync.dma_start(out=t[:], in_=ap[:])
            return t

        def loadc(ap, tag):
            rows, ncols = ap.shape
            k = rows // 128
            t = csb((128, k, ncols), ap.dtype, tag)
            srcap = bass.AP(tensor=ap.tensor, offset=0,
                            ap=[[ncols, 128], [128 * ncols, k], [1, ncols]])
            nc.sync.dma_start(out=t[:], in_=srcap)
            return t

        ident = load(identf_d, "ident")
        ones1 = load(ones1_d, "ones1")
        onesf = load(onesf_d, "onesf")
        peW1T = load(peW1T_d, "peW1T")
        bq = load(bq_d, "bq")
        bk = load(bk_d, "bk")
        ffb1 = load(ffb1_d, "ffb1")
        camsel = load(camsel_d, "camsel")
        chsel = load(chsel_d, "chsel")
        xpos_bf = load(xposbf_d, "xposbf")
        xv_bf = load(xvbf_d, "xvbf")
        xloc_bf = load(xlocbf_d, "xlocbf")
        qrows = loadc(qrows_d, "qrows")
        refrows = loadc(refrows_d, "refr")
        lidar = load(lidarT_d, "lidar")
        epst = csb((128, 1), F32, "epst")
        nc.vector.memset(epst[:], EPS)

        # ==================================================================
        # S1: projection -> pixel coords -> per-corner idx (int16) + coef
        # ==================================================================
        refs = load(refs_d, "refs")
        coords = csb((128, 8, 4 * NCAM), F32, "coords")
        for mt in range(8):
            rp = work.tile([4, 128], F32, name="rp", tag="rp")
            nc.vector.memset(rp[:], 1.0)
            nc.vector.tensor_copy(out=rp[0:3, :], in_=refs[:, ts_(mt, 128)])
            cp = ps_mm.tile([128, 4 * NCAM], F32, name='psmm')
            MM(cp[:], rp[:], lidar[:], start=True, stop=True)
            nc.vector.tensor_copy(out=coords[:, mt, :], in_=cp[:])

        shp = (128, 8, NCAM)

        def wtile(tag):
            return work.tile(list(shp), F32, name=tag, tag=tag)

        x_ = coords[:, :, 0::4]
        y_ = coords[:, :, 1::4]
        z_ = coords[:, :, 2::4]
        zc = wtile("zc")
        nc.vector.tensor_scalar_max(zc[:], z_, EPS)
        rz = csb(shp, F32, "rz")
        nc.vector.reciprocal(rz[:], zc[:])
        xr = csb(shp, F32, "xr")
        nc.vector.tensor_mul(xr[:], x_, rz[:])
        yr = csb(shp, F32, "yr")
        nc.vector.tensor_mul(yr[:], y_, rz[:])
        msk = csb(shp, F32, "msk")
        nc.vector.tensor_scalar(msk[:], z_, EPS, None, AL.is_gt)
        for src, cval, op in ((xr, 0.0, AL.is_gt), (xr, IMG_W, AL.is_lt),
                              (yr, 0.0, AL.is_gt), (yr, IMG_H, AL.is_lt)):
            t = wtile("mt")
            nc.vector.tensor_scalar(t[:], src[:], cval, None, op)
            nc.vector.tensor_mul(msk[:], msk[:], t[:])

        idxf = {}
        cofb = {}

        def cam_bcast(u):
            base = camsel[:, u, :]
            return bass.AP(tensor=base.tensor, offset=base.offset,
                           ap=[base.ap[0], [0, 8], base.ap[1]])

        i16pack = csb((128, NJ, 8), I16, "i16pack")
        for l in range(NL):
            Hl, Wl = LVLS[l]
            px = work.tile(list(shp), F32, name="px", tag="px", bufs=2)
            py = work.tile(list(shp), F32, name="py", tag="py", bufs=2)
            nc.vector.tensor_scalar(px[:], xr[:], Wl / IMG_W, -1.0,
                                    AL.mult, AL.add)
            nc.vector.tensor_scalar(py[:], yr[:], Hl / IMG_H, -1.0,
                                    AL.mult, AL.add)
            nc.vector.tensor_scalar(px[:], px[:], -4.0, Wl + 4.0,
                                    AL.max, AL.min)
            nc.vector.tensor_scalar(py[:], py[:], -4.0, Hl + 4.0,
                                    AL.max, AL.min)

            def corner_axis(pc, W, ax):
                i16 = work.tile(list(shp), I16, name="ci", tag=f"ci{ax}")
                nc.vector.tensor_copy(out=i16[:], in_=pc[:])
                c0f = cons.tile(list(shp), F32, name="c0f",
                                tag=f"c0f{ax}", bufs=2)
                nc.vector.tensor_copy(out=c0f[:], in_=i16[:])
                w1 = cons.tile(list(shp), F32, name="w1", tag=f"w1{ax}", bufs=2)
                nc.vector.scalar_tensor_tensor(
                    out=w1[:], in0=pc[:], scalar=0.5, in1=c0f[:],
                    op0=AL.add, op1=AL.subtract)
                w0 = cons.tile(list(shp), F32, name="w0", tag=f"w0{ax}", bufs=2)
                nc.vector.tensor_scalar(w0[:], w1[:], -1.0, 1.0,
                                        AL.mult, AL.add)
                v0 = cons.tile(list(shp), F32, name="v0", tag=f"v0{ax}", bufs=2)
                v1 = cons.tile(list(shp), F32, name="v1", tag=f"v1{ax}", bufs=2)
                t = wtile(f"vt{l}{ax}")
                nc.vector.tensor_scalar(v0[:], c0f[:], 0.0, None, AL.is_ge)
                nc.vector.tensor_scalar(t[:], c0f[:], float(W - 1), None,
                                        AL.is_le)
                nc.vector.tensor_mul(v0[:], v0[:], t[:])
                nc.vector.tensor_scalar(v1[:], c0f[:], -1.0, None, AL.is_ge)
                nc.vector.tensor_scalar(t[:], c0f[:], float(W - 2), None,
                                        AL.is_le)
                nc.vector.tensor_mul(v1[:], v1[:], t[:])
                cc0 = cons.tile(list(shp), F32, name="cc0", tag=f"cc0{ax}", bufs=2)
                cc1 = cons.tile(list(shp), F32, name="cc1", tag=f"cc1{ax}", bufs=2)
                nc.vector.tensor_scalar(cc0[:], c0f[:], 0.0, float(W - 1),
                                        AL.max, AL.min)
                nc.vector.tensor_scalar(cc1[:], c0f[:], 1.0, float(W - 1),
                                        AL.add, AL.min)
                nc.vector.tensor_scalar_max(cc1[:], cc1[:], 0.0)
                return (cc0, cc1), (w0, w1), (v0, v1)

            (xc0, xc1), (wx0, wx1), (vx0, vx1) = corner_axis(px, Wl, "x")
            (yc0, yc1), (wy0, wy1), (vy0, vy1) = corner_axis(py, Hl, "y")
            vwy, vwx, yrow = [], [], []
            for ci, (wc, vc) in enumerate(((wy0, vy0), (wy1, vy1))):
                t = cons.tile(list(shp), F32, name="vwy", tag=f"vwy{ci}", bufs=2)
                nc.vector.tensor_mul(t[:], wc[:], vc[:])
                nc.vector.tensor_mul(t[:], t[:], msk[:])
                vwy.append(t)
            for ci, (wc, vc) in enumerate(((wx0, vx0), (wx1, vx1))):
                t = cons.tile(list(shp), F32, name="vwx", tag=f"vwx{ci}", bufs=2)
                nc.vector.tensor_mul(t[:], wc[:], vc[:])
                vwx.append(t)
            for ci, yc in enumerate((yc0, yc1)):
                t = cons.tile(list(shp), F32, name="yrow", tag=f"yrow{ci}", bufs=2)
                nc.vector.tensor_scalar_mul(t[:], yc[:], float(Wl))
                yrow.append(t)
            for cy in range(2):
                for cx in range(2):
                    ti = cons.tile(list(shp), F32, name="idxf", tag=f"idxf{cy}{cx}", bufs=2)
                    nc.vector.tensor_add(ti[:], yrow[cy][:],
                                         (xc0, xc1)[cx][:])
                    idxf[(l, cy, cx)] = ti
                    tcf = csb(shp, F32, f"cofb{l}{cy}{cx}")
                    nc.vector.tensor_mul(tcf[:], vwy[cy][:], vwx[cx][:])
                    cofb[(l, cy, cx)] = tcf
            for u in range(NU):
                for cy in range(2):
                    for cx in range(2):
                        j = (u * NL + l) * 4 + 2 * cy + cx
                        t = wtile("selm")
                        nc.vector.tensor_mul(t[:], idxf[(l, cy, cx)][:],
                                             cam_bcast(u))
                        red = work.tile([128, 8], F32, name="red", tag="red")
                        nc.vector.tensor_reduce(red[:], t[:], AX.X, AL.add)
                        nc.vector.tensor_copy(out=i16pack[:, j, :],
                                              in_=red[:])

        # --- per-unit camera select of indices, int16 pack, DRAM shuffle ---
        # store to DRAM in gather layout: value i=(cn*1024+q) at
        # [i%16, jblk*64 + i//16], q = b*128 + p
        for ph in range(8):
            src = i16pack[ph * 16:(ph + 1) * 16, :, :]
            dst = bass.AP(tensor=idx_d.tensor, offset=ph,
                          ap=[[NJ * 64, 16], [64, NJ], [8, 8]])
            nc.sync.dma_start(out=dst, in_=src)
        idxrep = csb((128, NJ * 64), I16, "idxrep")
        for r in range(8):
            nc.sync.dma_start(out=idxrep[r * 16:(r + 1) * 16, :],
                              in_=idx_d[:])

        if FLAGS["dbg"]:
            nc.sync.dma_start(out=dbg_idx[:], in_=idxrep[:])
        # ==================================================================
        # S2: QKV projections (bf16)
        # ==================================================================
        wqkvT = lw(wqkvT_d)
        bv_b = lb(bv_d)
        qhT = csb((128, 2, QL), BF, "qhT")   # head h -> [32*(h%4), h//4, :]
        khT = csb((128, 2, QP), BF, "khT")
        for ch in range(2):
            qp_ = ps_mm.tile([128, QL], F32, name='psmm')
            for kc in range(2):
                MM(qp_[:], wqkvT[:, kc, ts_(ch, 128)],
                   xloc_bf[:, kc, :], start=(kc == 0), stop=(kc == 1))
            nc.vector.tensor_scalar(qhT[:, ch, :], qp_[:],
                                    bq[:, ch:ch + 1], None, AL.add)
            for nn_ in range(2):
                kp = ps_mm.tile([128, 512], F32, name='psmm')
                for kc in range(2):
                    MM(kp[:], wqkvT[:, kc, 256 + ch * 128:256 + (ch + 1) * 128],
                       xpos_bf[:, kc, ts_(nn_, 512)],
                       start=(kc == 0), stop=(kc == 1))
                nc.vector.tensor_scalar(khT[:, ch, ts_(nn_, 512)], kp[:],
                                        bk[:, ch:ch + 1], None, AL.add)
        v_sb = []
        for kt in range(8):
            vt = csb((128, NH, DH + 1), BF, f"vsb{kt}")
            nc.vector.memset(vt[:], 1.0)
            v_sb.append(vt)
        for kt in range(8):
            vp = ps_mm.tile([128, C], F32, name='psmm')
            for kc in range(2):
                MM(vp[:], xv_bf[:, kc, ts_(kt, 128)],
                   wqkvT[:, kc, 512:768],
                   start=(kc == 0), stop=(kc == 1))
            vt = v_sb[kt]
            dst = bass.AP(tensor=vt.tensor, offset=vt[:].offset,
                          ap=[vt[:].ap[0], [DH + 1, NH], [1, DH]])
            nc.vector.tensor_add(dst, vp[:], bv_b[:])

        # ==================================================================
        # S3: attention per head -> oT
        # ==================================================================
        oT = csb((128, 2, QL), BF, "oT")
        for h in range(NH):
            r, chh = 32 * (h % 4), h // 4
            esc = esc_p.tile([128, 8, QL], BF, name="esc", tag="esc")
            for kt in range(8):
                sp = ps_mm.tile([128, QL], F32, name='psmm')
                MM(sp[:], khT[r:r + 32, chh, ts_(kt, 128)],
                   qhT[r:r + 32, chh, :], start=True, stop=True,
                   tile_position=(r, 0))
                if kt == 7:
                    nc.vector.memset(esc[:, 7, :], 0.0)
                    nc.scalar.activation(esc[0:4, 7, :], sp[0:4, :], AF.Exp)
                else:
                    nc.scalar.activation(esc[:, kt, :], sp[:], AF.Exp)
            ov = ps_mm.tile([DH + 1, QL], F32, name='psmm')
            for kt in range(8):
                MM(ov[:], v_sb[kt][:, h, :], esc[:, kt, :],
                   start=(kt == 0), stop=(kt == 7))
            ovs = work.tile([DH + 1, QL], F32, name="ovs", tag="ovs", bufs=2)
            nc.scalar.copy(out=ovs[:], in_=ov[:])
            rinv = work.tile([1, QL], F32, name="rinv", tag="rinv", bufs=2)
            nc.vector.reciprocal(rinv[:], ovs[DH:DH + 1, :])
            bc = ps_bc.tile([DH, QL], F32, name='psbc')
            MM(bc[:], onesf[:, 0:DH], rinv[:], start=True, stop=True)
            nc.vector.tensor_mul(oT[r:r + 32, chh, :], ovs[0:DH, :], bc[:])

        # S4: sa rows + residual + LN1 -> x1 rows; x1T
        x1 = csb((128, 2, C), F32, "x1")

        def layernorm(dst, pre, g_d, b_d):
            g = lb(g_d)
            b = lb(b_d)
            st = work.tile([128, 6], F32, name="lnst", tag="lnst")
            nc.vector.bn_stats(out=st[:], in_=pre)
            mv = work.tile([128, 2], F32, name="lnmv", tag="lnmv")
            nc.vector.bn_aggr(out=mv[:], in_=st[:])
            sd = work.tile([128, 1], F32, name="lnsd", tag="lnsd")
            nc.scalar.activation(sd[:], mv[:, 1:2], AF.Sqrt, bias=epst[:])
            ri = work.tile([128, 1], F32, name="lnri", tag="lnri")
            nc.vector.reciprocal(ri[:], sd[:])
            nc.vector.tensor_scalar_sub(dst, pre, mv[:, 0:1])
            nc.vector.tensor_scalar_mul(dst, dst, ri[:])
            nc.vector.tensor_mul(dst, dst, g[:])
            nc.vector.tensor_add(dst, dst, b[:])

        woT = lw(woT_d)
        bo_b = lb(bo_d)
        for m in range(2):
            sap = ps_mm.tile([128, C], F32, name='psmm')
            for kc in range(2):
                MM(sap[:], oT[:, kc, ts_(m, 128)], woT[:, kc, :],
                   start=(kc == 0), stop=(kc == 1))
            t = work.tile([128, C], F32, name="sar", tag="sar", bufs=1)
            nc.vector.tensor_add(t[:], sap[:], bo_b[:])
            nc.vector.tensor_add(t[:], t[:], qrows[:, m, :])
            layernorm(x1[:, m, :], t[:], nrm_d["n1g"], nrm_d["n1b"])

        x1T = csb((128, 2, QL), BF, "x1T")
        for m in range(2):
            for cc in range(2):
                tp = ps_tp.tile([128, 128], F32, name='pstp')
                nc.tensor.transpose(tp[:], x1[:, m, ts_(cc, 128)], ident[:])
                nc.scalar.copy(out=x1T[:, cc, ts_(m, 128)], in_=tp[:])

        # S5: aw rows -> AllGather -> coef pack -> DRAM rows
        attWT = lw(attWT_d)
        attb_b = lb(attb_d)
        awr = csb((128, 2, AWC), F32, "awr")
        for m in range(2):
            ap_ = ps_mm.tile([128, AWC], F32, name='psmm')
            for kc in range(2):
                MM(ap_[:], x1T[:, kc, ts_(m, 128)], attWT[:, kc, :],
                   start=(kc == 0), stop=(kc == 1))
            t = work.tile([128, AWC], F32, name="awt", tag="awt")
            nc.vector.tensor_add(t[:], ap_[:], attb_b[:])
            nc.scalar.activation(awr[:, m, :], t[:], AF.Sigmoid)
            nc.sync.dma_start(out=aw_in[ts_(m, 128), :], in_=awr[:, m, :])
        if FLAGS["coll"]:
            nc.gpsimd.collective_compute(
                "AllGather", AL.bypass, replica_groups=GROUPS,
                ins=[aw_in[:]], outs=[aw_all[:]])
        else:
            for _r in range(4):
                nc.sync.dma_start(out=aw_all[_r * QL:(_r + 1) * QL, :],
                                  in_=aw_in[:])
        aw_sb = csb((128, 8, AWC), F32, "awsb")
        aw_src = bass.AP(tensor=aw_all.tensor, offset=0,
                         ap=[[AWC, 128], [128 * AWC, 8], [1, AWC]])
        nc.sync.dma_start(out=aw_sb[:], in_=aw_src)

        cofpack = csb((128, NJ, 8), BF, "cofpack")
        for l in range(NL):
            base = aw_sb[:, :, l:l + 1]
            aw_l = bass.AP(tensor=base.tensor, offset=base.offset,
                           ap=[base.ap[0], [AWC, 8], [NL, NCAM]])
            for cy in range(2):
                for cx in range(2):
                    cf = work.tile(list(shp), F32, name="cfin", tag="cfin")
                    nc.vector.tensor_mul(cf[:], cofb[(l, cy, cx)][:], aw_l)
                    for u in range(NU):
                        j = (u * NL + l) * 4 + 2 * cy + cx
                        t = wtile("cselm")
                        nc.vector.tensor_mul(t[:], cf[:], cam_bcast(u))
                        red = work.tile([128, 8], F32, name="red2", tag="red2")
                        nc.vector.tensor_reduce(red[:], t[:], AX.X, AL.add)
                        nc.vector.tensor_copy(out=cofpack[:, j, :],
                                              in_=red[:])
        # coef rows to DRAM: row (u*NL+l), col = cn*1024 + b*128 + p
        for ul in range(NU * NL):
            src = cofpack[:, ul * 4:(ul + 1) * 4, :]
            dst = bass.AP(tensor=cof_d.tensor, offset=ul * 4096,
                          ap=[[1, 128], [1024, 4], [128, 8]])
            nc.sync.dma_start(out=dst, in_=src)

        if FLAGS["dbg"]:
            for _ul in range(NJ // 4):
                nc.sync.dma_start(out=dbg_cof[_ul:_ul + 1, :],
                                  in_=cof_d[_ul:_ul + 1, :])
        # ==================================================================
        # S6: feature sweep + gather + combine
        # ==================================================================
        acc = csb((128, 2, QP), F32, "acc")
        nc.vector.memset(acc[:], 0.0)
        galt = [0]
        for u in range(NU):
            for l in range(NL):
                hw, nblk = HWL[l], BLKS[l]
                src = feat_d[l][u].rearrange("c h w -> c (h w)")
                pyr = pyr_p[l].tile([128, nblk * 128], BF, name=f"pyr{l}", tag=f"pyr{l}")
                col = 0
                while col < (hw if FLAGS["sweep"] else 0):
                    w = min(1024, hw - col)
                    nb = (w + 127) // 128
                    st = stage_p.tile([128, 1024], F32, name="st", tag="st")
                    nc.sync.dma_start(out=st[:, :w], in_=src[:, col:col + w])
                    for b_ in range(nb):
                        gb = col // 128 + b_
                        tp = ps_tp.tile([128, 128], F32, name='pstp')
                        nc.tensor.transpose(tp[:], st[:, ts_(b_, 128)],
                                            ident[:])
                        if (galt[0] * 4) % 9 < 4:
                            nc.vector.tensor_copy(out=pyr[:, ts_(gb, 128)],
                                                  in_=tp[:])
                        else:
                            nc.scalar.copy(out=pyr[:, ts_(gb, 128)],
                                           in_=tp[:])
                        galt[0] += 1
                    col += w
                jb = (u * NL + l) * 4
                for half in range(2 if FLAGS["gather"] else 0):
                    g = g_p.tile([128, 1, 2048], BF, name="g", tag="g")
                    nc.gpsimd.dma_gather(
                        out_ap=g[:], in_ap=pyr[:],
                        idxs_ap=idxrep[:, (jb + 2 * half) * 64:
                                       (jb + 2 * half + 2) * 64],
                        num_idxs=2048, num_idxs_reg=2048, elem_size=128,
                        transpose=True, sbuf_tokens_per_rank=128,
                        sbuf_free_dim_per_rank=256, single_packet=False)
                    for cnh in range(2 if FLAGS["comb"] else 0):
                        cn = 2 * half + cnh
                        crow = crow_p.tile([1, 1024], BF, name="crow",
                                           tag="crow")
                        nc.sync.dma_start(
                            out=crow[:],
                            in_=cof_d[u * NL + l:u * NL + l + 1,
                                      cn * 1024:(cn + 1) * 1024])
                        for qc in range(2):
                            s0 = cnh * 1024 + qc * 512
                            bcp = ps_bc.tile([128, 512], F32, name='psbc')
                            MM(bcp[:], ones1[:],
                               crow[:, qc * 512:(qc + 1) * 512],
                               start=True, stop=True)
                            tmp = tmp_p.tile([128, 512], F32, name="tmp",
                                             tag="tmp")
                            nc.vector.tensor_mul(tmp[:], g[:, 0, s0:s0 + 512],
                                                 bcp[:])
                            for chn in range(2):
                                eng = nc.vector
                                eng.scalar_tensor_tensor(
                                    out=acc[:, chn, ts_(qc, 512)], in0=tmp[:],
                                    scalar=chsel[:, u, chn:chn + 1],
                                    in1=acc[:, chn, ts_(qc, 512)],
                                    op0=AL.mult, op1=AL.add)

        # acc (c,q) -> rows (q,c) -> ReduceScatter
        for qh in range(2):
            frh = work.tile([128, 4, C], F32, name="frh", tag="frh", bufs=1)
            for qb4 in range(4):
                qb = qh * 4 + qb4
                for chn in range(2):
                    tp = ps_tp.tile([128, 128], F32, name='pstp')
                    nc.tensor.transpose(tp[:], acc[:, chn, ts_(qb, 128)],
                                        ident[:])
                    nc.scalar.copy(out=frh[:, qb4, ts_(chn, 128)], in_=tp[:])
            fsdst = bass.AP(tensor=fs_in.tensor, offset=qh * 4 * 128 * C,
                            ap=[[C, 128], [128 * C, 4], [1, C]])
            nc.sync.dma_start(out=fsdst, in_=frh[:])
        if FLAGS["coll"]:
            nc.gpsimd.collective_compute(
                "ReduceScatter", AL.add, replica_groups=GROUPS,
                ins=[fs_in[:]], outs=[fs_out[:]])
        else:
            nc.sync.dma_start(out=fs_out[:], in_=fs_in[0:QL, :])

        # ==================================================================
        # S7: tail
        # ==================================================================
        fr = csb((128, 2, C), F32, "fr")
        frsrc = bass.AP(tensor=fs_out.tensor, offset=0,
                        ap=[[C, 128], [128 * C, 2], [1, C]])
        nc.sync.dma_start(out=fr[:], in_=frsrc)
        fT = csb((128, 2, QL), BF, "fT")
        for m in range(2):
            for cc in range(2):
                tp = ps_tp.tile([128, 128], F32, name='pstp')
                nc.tensor.transpose(tp[:], fr[:, m, ts_(cc, 128)], ident[:])
                nc.scalar.copy(out=fT[:, cc, ts_(m, 128)], in_=tp[:])

        # position encoder
        iref = csb((128, 2, 3), F32, "iref")
        for m in range(2):
            rr = refrows[:, m, :]
            a = work.tile([128, 3], F32, name="pea", tag="pea")
            b2 = work.tile([128, 3], F32, name="peb2t", tag="peb2t")
            nc.vector.tensor_scalar(a[:], rr, EPS, 1.0, AL.max, AL.min)
            nc.vector.tensor_scalar(b2[:], rr, -1.0, 1.0, AL.mult, AL.add)
            nc.vector.tensor_scalar(b2[:], b2[:], EPS, 1.0, AL.max, AL.min)
            rb = work.tile([128, 3], F32, name="perb", tag="perb")
            nc.vector.reciprocal(rb[:], b2[:])
            nc.vector.tensor_mul(a[:], a[:], rb[:])
            nc.scalar.activation(iref[:, m, :], a[:], AF.Ln)
        irT = csb((3, QL), BF, "irT")
        for m in range(2):
            tp = ps_tp.tile([128, 128], F32, name='pstp')
            nc.tensor.transpose(tp[0:3, :], iref[:, m, :], ident[:])
            nc.scalar.copy(out=irT[:, ts_(m, 128)], in_=tp[0:3, :])
        peb1_b = lb(peb1_d)
        pe1 = csb((128, 2, C), F32, "pe1")
        for m in range(2):
            pp = ps_mm.tile([128, C], F32, name='psmm')
            MM(pp[:], irT[:, ts_(m, 128)], peW1T[:], start=True, stop=True)
            t = work.tile([128, C], F32, name="pet", tag="pet", bufs=1)
            nc.vector.tensor_add(t[:], pp[:], peb1_b[:])
            layernorm(pe1[:, m, :], t[:], nrm_d["peg1"], nrm_d["pebt1"])
            nc.scalar.activation(pe1[:, m, :], pe1[:, m, :], AF.Relu)
        pe1T = csb((128, 2, QL), BF, "pe1T")
        for m in range(2):
            for cc in range(2):
                tp = ps_tp.tile([128, 128], F32, name='pstp')
                nc.tensor.transpose(tp[:], pe1[:, m, ts_(cc, 128)], ident[:])
                nc.scalar.copy(out=pe1T[:, cc, ts_(m, 128)], in_=tp[:])

        outWT = lw(outWT_d)
        peW2T = lw(peW2T_d)
        outb_b = lb(outb_d)
        peb2_b = lb(peb2_d)
        x2 = csb((128, 2, C), F32, "x2")
        for m in range(2):
            op_ = ps_mm.tile([128, C], F32, name='psmm')
            for kc in range(2):
                MM(op_[:], fT[:, kc, ts_(m, 128)], outWT[:, kc, :],
                   start=(kc == 0), stop=(kc == 1))
            pp = ps_mm.tile([128, C], F32, name='psmm')
            for kc in range(2):
                MM(pp[:], pe1T[:, kc, ts_(m, 128)], peW2T[:, kc, :],
                   start=(kc == 0), stop=(kc == 1))
            pe2 = work.tile([128, C], F32, name="pe2", tag="pe2", bufs=1)
            nc.vector.tensor_add(pe2[:], pp[:], peb2_b[:])
            layernorm(pe2[:], pe2[:], nrm_d["peg2"], nrm_d["pebt2"])
            nc.scalar.activation(pe2[:], pe2[:], AF.Relu)
            cr = work.tile([128, C], F32, name="cr", tag="cr", bufs=1)
            nc.vector.tensor_add(cr[:], op_[:], outb_b[:])
            nc.vector.tensor_add(cr[:], cr[:], pe2[:])
            nc.vector.tensor_add(cr[:], cr[:], x1[:, m, :])
            layernorm(x2[:, m, :], cr[:], nrm_d["n2g"], nrm_d["n2b"])

        x2T = csb((128, 2, QL), BF, "x2T")
        for m in range(2):
            for cc in range(2):
                tp = ps_tp.tile([128, 128], F32, name='pstp')
                nc.tensor.transpose(tp[:], x2[:, m, ts_(cc, 128)], ident[:])
                nc.scalar.copy(out=x2T[:, cc, ts_(m, 128)], in_=tp[:])

        # FFN
        ffW1T = lw(ffW1T_d)
        ffW2T = lw(ffW2T_d)
        ffb2_b = lb(ffb2_d)
        yps = [ps_mm.tile([128, C], F32, name='psmm') for _ in range(2)]
        for ft in range(8):
            hp = ps_mm.tile([128, QL], F32, name='psmm')
            for kc in range(2):
                MM(hp[:], ffW1T[:, kc, ts_(ft, 128)], x2T[:, kc, :],
                   start=(kc == 0), stop=(kc == 1))
            h_ = ht_p.tile([128, QL], BF, name="hT", tag="hT")
            nc.scalar.activation(h_[:], hp[:], AF.Relu,
                                 bias=ffb1[:, ft:ft + 1])
            for m in range(2):
                MM(yps[m][:], h_[:, ts_(m, 128)], ffW2T[:, ft, :],
                   start=(ft == 0), stop=(ft == 7))
        for m in range(2):
            t = work.tile([128, C], F32, name="yt", tag="yt", bufs=1)
            nc.vector.tensor_add(t[:], yps[m][:], ffb2_b[:])
            nc.vector.tensor_add(t[:], t[:], x2[:, m, :])
            o = work.tile([128, C], F32, name="orow", tag="orow", bufs=1)
            layernorm(o[:], t[:], nrm_d["n3g"], nrm_d["n3b"])
            nc.sync.dma_start(out=out_rows[ts_(m, 128), :], in_=o[:])


# ==========================================================================
# Host side
# ==========================================================================

def _host_inputs(inputs):
    import ml_dtypes
    f32 = np.float32
    bf16 = ml_dtypes.bfloat16

    query = np.asarray(inputs["query"], f32)
    query_pos = np.asarray(inputs["query_pos"], f32)
    ref = np.asarray(inputs["reference_points"], f32)
    lidar = np.asarray(inputs["lidar2img"], f32)
    feats = [np.asarray(inputs[f"feat{l}"], f32) for l in range(NL)]

    xpos = query + query_pos
    scale = 1.0 / math.sqrt(DH)
    Wqkv = np.asarray(inputs["Wqkv"], f32).copy()
    bqkv = np.asarray(inputs["bqkv"], f32)
    Wqkv[:C] *= scale

    def bcastp(v, n=C):
        return np.ascontiguousarray(
            np.broadcast_to(np.asarray(v, f32).reshape(-1)[:n], (128, n)))

    def colmaj(v, ncols):
        return np.ascontiguousarray(np.asarray(v, f32).reshape(ncols, 128).T)

    common = dict(
        wqkvT=np.ascontiguousarray(Wqkv.T).astype(bf16),
        woT=np.ascontiguousarray(np.asarray(inputs["Wo"], f32).T).astype(bf16),
        attWT=np.ascontiguousarray(
            np.asarray(inputs["attW"], f32).T).astype(bf16),
        outWT=np.ascontiguousarray(
            np.asarray(inputs["outW"], f32).T).astype(bf16),
        peW1T=np.ascontiguousarray(
            np.asarray(inputs["peW1"], f32).T).astype(bf16),
        peW2T=np.ascontiguousarray(
            np.asarray(inputs["peW2"], f32).T).astype(bf16),
        ffW1T=np.ascontiguousarray(
            np.asarray(inputs["ffW1"], f32).T).astype(bf16),
        ffW2T=np.ascontiguousarray(
            np.asarray(inputs["ffW2"], f32).T).astype(bf16),
        bq=colmaj(bqkv[:C] * scale, 2),
        bk=colmaj(bqkv[C:2 * C], 2),
        bv_b=bcastp(bqkv[2 * C:]),
        bo_b=bcastp(inputs["bo"]),
        attb_b=bcastp(inputs["attb"], NCAM * NL),
        outb_b=bcastp(inputs["outb"]),
        peb1_b=bcastp(inputs["peb1"]),
        peb2_b=bcastp(inputs["peb2"]),
        ffb1=colmaj(inputs["ffb1"], 8),
        ffb2_b=bcastp(inputs["ffb2"]),
        identf=np.eye(128, dtype=f32),
        ones1=np.ones((1, 128), bf16),
        onesf=np.ones((1, 128), f32),
    )
    for nm in ("n1g", "n1b", "n2g", "n2b", "n3g", "n3b",
               "peg1", "pebt1", "peg2", "pebt2"):
        common[nm + "_b"] = bcastp(inputs[nm])

    def pad_q(a, axis):
        pad = [(0, 0)] * a.ndim
        pad[axis] = (0, QP - a.shape[axis])
        return np.pad(a, pad)

    in_maps = []
    for core in range(NCORES):
        b, j = core // 4, core % 4
        units = [(3 * j + i) // 2 for i in range(NU)]
        chunks = [(3 * j + i) % 2 for i in range(NU)]
        camsel = np.zeros((128, NU, NCAM), f32)
        chsel = np.zeros((128, NU, 2), f32)
        for i in range(NU):
            camsel[:, i, units[i]] = 1.0
            chsel[:, i, chunks[i]] = 1.0
        m = dict(common)
        def chunk3(a):  # (256, N) -> (128, 2, N)
            return np.ascontiguousarray(
                a.reshape(2, 128, a.shape[1]).transpose(1, 0, 2))
        import ml_dtypes as _md
        m["xposbf"] = chunk3(pad_q(xpos[b].T, 1)).astype(_md.bfloat16)
        m["xvbf"] = chunk3(pad_q(query[b].T, 1)).astype(_md.bfloat16)
        m["xlocbf"] = chunk3(pad_q(xpos[b].T, 1)[:, j * QL:(j + 1) * QL]) \
            .astype(_md.bfloat16)
        m["qrows_loc"] = np.ascontiguousarray(
            pad_q(query[b], 0)[j * QL:(j + 1) * QL])
        m["refs"] = np.ascontiguousarray(
            pad_q(ref[b].T, 1) * np.array(PCS, np.float32).reshape(3, 1)
            + np.array(PCO, np.float32).reshape(3, 1))
        m["refrows_loc"] = np.ascontiguousarray(
            pad_q(ref[b], 0)[j * QL:(j + 1) * QL])
        m["lidarT"] = np.ascontiguousarray(np.concatenate(
            [np.ascontiguousarray(lidar[b, n].T) for n in range(NCAM)],
            axis=1))
        m["camsel"] = camsel
        m["chsel"] = chsel
        for l in range(NL):
            fp = np.stack(
                [feats[l][b, units[i],
                          chunks[i] * 128:(chunks[i] + 1) * 128]
                 for i in range(NU)], axis=0)
            m[f"feat{l}"] = np.ascontiguousarray(fp)
        in_maps.append(m)
    return in_maps


def kernel(**inputs):
    if "nc" not in _BUILT:
        _BUILT["nc"] = _build()
    nc = _BUILT["nc"]
    from concourse import bass_utils
    in_maps = _host_inputs(inputs)
    res = bass_utils.run_bass_kernel_spmd(nc, in_maps,
                                          core_ids=list(range(NCORES)))
    out = np.zeros((B, Q, C), np.float32)
    for core in range(NCORES):
        b, j = core // 4, core % 4
        rows = np.asarray(res.results[core]["out_rows"], np.float32)
        lo = j * QL
        hi = min((j + 1) * QL, Q)
        if lo < Q:
            out[b, lo:hi] = rows[:hi - lo]
    return out

